# revision 1
# baseline (speedup 1.0000x reference)
"""2-layer GAT (nn_GATModel) on 8 Trainium2 NeuronCores — v2.

Strategy (v2)
-------------
- Dst-node sharding with HOST-BALANCED node->(core,tile) assignment (LPT on
  in-degree) to minimize chunk padding; output inverse-permuted on host.
- Self-loop edges never gathered: their contribution is computed from
  locally-recomputed h1/h2 rows (phase X / t2keep) and added after
  aggregation.
- Phase A consumes host-transposed bf16 x (no on-device transposes), runs
  bf16 matmuls, and writes T1 rows (256 feat bf16 + 8 a_src f32, stride
  768B) in 8-tile batched DMAs.  T1 split into lo/hi DRAM tensors so the
  first low-half gathers can start while high tiles still compute.
- Phase B: per-edge rows gathered by SWDGE (idx16 lo/hi), one-hot Se/SeT
  built in ONE vector op per group, per-chunk PE matmuls for a_dst and
  aggregation (+ softmax denominator via a w column), dense layer-2
  transform fused per tile.
- Phase D: same structure on the AllGathered T2 table (256B rows).
"""

import sys

sys.path.insert(0, "/opt/trn_rl_repo")

import heapq
import math
from contextlib import ExitStack

import numpy as np
import ml_dtypes

import concourse.bass as bass  # noqa: F401
import concourse.tile as tile
from concourse import bacc, mybir
from concourse.bass_utils import run_bass_kernel_spmd
from concourse.masks import make_identity

P = 128
NEG_SLOPE = 0.2
EPS = 1e-16
F32 = mybir.dt.float32
BF16 = mybir.dt.bfloat16
I16 = mybir.dt.int16
BF = ml_dtypes.bfloat16

REAL = dict(
    n=50000, ncore=8, half=32768, heads=8, cdim=32, in_ch=128,
    cap_chunks=40, row1=384, row2=128,
)


# ---------------------------------------------------------------- host prep
def assign_nodes(deg, n, ncore, sh):
    """LPT-balance nodes into (core, tile) bins; return nodes_at, pos_of."""
    ntile = math.ceil(sh / P)
    caps = [P] * (ntile - 1) + [sh - P * (ntile - 1)]
    nbins = ncore * ntile
    order = np.argsort(-deg, kind="stable")
    heap = [(0, b) for b in range(nbins)]
    heapq.heapify(heap)
    fill = [[] for _ in range(nbins)]
    stash = []
    for v in order:
        while True:
            s, b = heapq.heappop(heap)
            if len(fill[b]) < caps[b % ntile]:
                break
            stash.append(None)  # bin full; drop permanently
        fill[b].append(v)
        if len(fill[b]) < caps[b % ntile]:
            heapq.heappush(heap, (s + int(deg[v]), b))
    nodes_at = np.empty(n, np.int64)
    pos_of = np.empty(n, np.int64)
    for b in range(nbins):
        k, t = divmod(b, ntile)
        base = k * sh + t * P
        for i, v in enumerate(fill[b]):
            nodes_at[base + i] = v
            pos_of[v] = base + i
    return nodes_at, pos_of, ntile


def build_schedule(src, dst, p):
    n, ncore, half = p["n"], p["ncore"], p["half"]
    sh = n // ncore
    deg = np.bincount(dst, minlength=n)
    nodes_at, pos_of, ntile = assign_nodes(deg, n, ncore, sh)

    sp = pos_of[src]
    dp = pos_of[dst]
    core = dp // sh
    within = dp % sh
    tl = within >> 7
    dloc = within & 127
    hi = (sp >= half).astype(np.int64)
    key = ((core * ntile + tl) * 2 + hi)
    order = np.argsort(key, kind="stable")
    sp, key = sp[order], key[order]
    dloc_s = dloc[order]
    hi_s = hi[order]

    bounds = np.searchsorted(key, np.arange(ncore * ntile * 2 + 1))
    counts = (bounds[1:] - bounds[:-1]).reshape(ncore, ntile, 2)
    cmax = counts.max(axis=0)                       # [ntile, 2]
    csched = np.maximum(-(-cmax // P), 1)

    groups = []
    cur = []
    for t in range(ntile):
        ct = int(csched[t].sum())
        if cur and sum(int(csched[x].sum()) for x in cur) + ct > p["cap_chunks"]:
            groups.append(cur)
            cur = []
        cur.append(t)
    groups.append(cur)

    chunk_tile = []
    group_info = []
    tile_info = {t: {} for t in range(ntile)}
    c0 = 0
    for g in groups:
        glow = sum(int(csched[t][0]) for t in g)
        ghigh = sum(int(csched[t][1]) for t in g)
        gi = dict(tiles=g, c0=c0, clow=glow, chigh=ghigh, ct=glow + ghigh)
        cc = c0
        for half_id in (0, 1):
            for t in g:
                nch = int(csched[t][half_id])
                tile_info[t]["low" if half_id == 0 else "high"] = (cc, nch)
                for _ in range(nch):
                    chunk_tile.append(t)
                cc += nch
        c0 = cc
        group_info.append(gi)
    nchunks = c0

    sched = dict(
        sh=sh, ntile=ntile, csched=csched, groups=group_info,
        chunk_tile=np.array(chunk_tile), tile_info=tile_info, nchunks=nchunks,
        tile_rows=[min(P, sh - t * P) for t in range(ntile)],
    )

    percore = []
    for k in range(ncore):
        idxs = np.zeros(nchunks * P, np.int16)
        dla = np.full(nchunks * P, 999.0, np.float32)
        for t in range(ntile):
            for half_id, kkey in ((0, "low"), (1, "high")):
                b0 = bounds[(k * ntile + t) * 2 + half_id]
                b1 = bounds[(k * ntile + t) * 2 + half_id + 1]
                vals = sp[b0:b1] - (half if half_id else 0)
                dv = dloc_s[b0:b1]
                cstart, cnum = tile_info[t][kkey]
                s0 = cstart * P
                idxs[s0 : s0 + len(vals)] = vals.astype(np.int16)
                dla[s0 : s0 + len(vals)] = dv.astype(np.float32)
        idx16 = np.tile(np.ascontiguousarray(idxs.reshape(-1, 16).T), (8, 1))
        dstcol = np.ascontiguousarray(
            dla.reshape(nchunks, P).T.astype(np.float32))
        dstrow = dla.reshape(1, -1).astype(BF)
        percore.append(dict(idx16=idx16, dstcol=dstcol, dstrow=dstrow))
    return sched, percore, nodes_at, pos_of


def prep_weights(W1, att_src1, att_dst1, bias1, W2, att_src2, att_dst2, p):
    H, C = p["heads"], p["cdim"]
    HC = H * C
    W1p = W1
    vsrc1 = np.einsum("khc,hc->kh", W1.reshape(-1, H, C), att_src1)
    vdst1 = np.einsum("khc,hc->kh", W1.reshape(-1, H, C), att_dst1)
    w1ev = np.concatenate([W1p, vsrc1, vdst1], axis=1)       # [128, 272]

    vsrc2 = W2 @ att_src2[0]
    vdst2 = W2 @ att_dst2[0]
    r2ext = np.concatenate(
        [W2, vsrc2[:, None], vdst2[:, None]], axis=1)        # [256, 34]
    r2b = r2ext.astype(BF)
    # csum must match the QUANTIZED weights: the elu+1 encoding relies on
    # exact cancellation of the +1 term (sum over 256 rows).
    csum = r2b.astype(np.float32).sum(axis=0)
    return (w1ev.astype(BF), r2b,
            csum.astype(np.float32)[None, :],
            bias1.astype(np.float32)[None, :])


# ---------------------------------------------------------------- device
def build_program(sched, p):
    n, ncore, half = p["n"], p["ncore"], p["half"]
    H, C, IC = p["heads"], p["cdim"], p["in_ch"]
    HC = H * C                                      # 256
    ROW1, ROW2 = p["row1"], p["row2"]               # 384, 128 (bf16 units)
    R2E = C + 2                                     # 34
    sh, ntile, nchunks = sched["sh"], sched["ntile"], sched["nchunks"]
    cap = p["cap_chunks"]
    natile = math.ceil(n / P)                       # 391
    nlo_t = half // P                               # 256 tiles in lo
    npos = natile * P                               # 50048
    BAT = 8

    nc = bacc.Bacc("TRN2", target_bir_lowering=False, debug=False,
                   num_devices=ncore)

    i_xT = nc.dram_tensor("xT", [IC, npos], BF16, kind="ExternalInput")
    i_xTmy = nc.dram_tensor("xTmy", [IC, sh], BF16, kind="ExternalInput")
    i_idx = nc.dram_tensor("idx16", [P, nchunks * 8], I16, kind="ExternalInput")
    i_dstcol = nc.dram_tensor("dstcol", [P, nchunks], F32, kind="ExternalInput")
    i_dstrow = nc.dram_tensor("dstrow", [1, nchunks * P], BF16,
                              kind="ExternalInput")
    i_w1ev = nc.dram_tensor("w1ev", [IC, HC + 2 * H], BF16,
                            kind="ExternalInput")
    i_r2 = nc.dram_tensor("r2ext", [HC, R2E], BF16, kind="ExternalInput")
    i_csum = nc.dram_tensor("csum", [1, R2E], F32, kind="ExternalInput")
    i_b1 = nc.dram_tensor("bias1", [1, HC], F32, kind="ExternalInput")
    i_b2 = nc.dram_tensor("bias2", [1, C], F32, kind="ExternalInput")
    o_out = nc.dram_tensor("out", [sh, C], F32, kind="ExternalOutput")

    d_t1lo = nc.dram_tensor("T1lo", [half, ROW1], BF16)
    d_t1hi = nc.dram_tensor("T1hi", [npos - half, ROW1], BF16)
    d_t2s = nc.dram_tensor("T2slab", [sh, ROW2], BF16)
    d_t2f = nc.dram_tensor("T2full", [n, ROW2], BF16,
                           addr_space="Shared" if ncore > 4 else "Local")

    AF = mybir.ActivationFunctionType
    OP = mybir.AluOpType

    with tile.TileContext(nc) as tc, ExitStack() as ctx:
        const = ctx.enter_context(tc.tile_pool(name="const", bufs=1))

        iota_col = const.tile([P, 1], F32)
        nc.gpsimd.iota(iota_col[:], pattern=[[0, 1]], base=0,
                       channel_multiplier=1,
                       allow_small_or_imprecise_dtypes=True)
        iota_rep = const.tile([P, cap * P], BF16)
        nc.gpsimd.iota(iota_rep[:], pattern=[[0, cap], [1, P]], base=0,
                       channel_multiplier=0,
                       allow_small_or_imprecise_dtypes=True)
        ident32 = const.tile([P, P], F32)
        make_identity(nc, ident32[:])
        identbf = const.tile([P, P], BF16)
        make_identity(nc, identbf[:])
        w1ev_sb = const.tile([IC, HC + 2 * H], BF16)
        nc.sync.dma_start(w1ev_sb[:], i_w1ev[:, :])
        r2_sb = const.tile([P, 2, R2E], BF16)
        for j in range(2):
            nc.sync.dma_start(r2_sb[:, j, :], i_r2[j * P : (j + 1) * P, :])
        csum_sb = const.tile([P, R2E], F32)
        nc.gpsimd.dma_start(csum_sb[:], i_csum[:, :].to_broadcast([P, R2E]))
        b1_sb = const.tile([P, HC], F32)
        nc.gpsimd.dma_start(b1_sb[:], i_b1[:, :].to_broadcast([P, HC]))
        b2_sb = const.tile([P, C], F32)
        nc.gpsimd.dma_start(b2_sb[:], i_b2[:, :].to_broadcast([P, C]))
        xTmy_sb = const.tile([IC, sh], BF16)
        nc.sync.dma_start(xTmy_sb[:], i_xTmy[:, :])
        adst1_sb = const.tile([P, ntile, H], BF16)
        nc.vector.memset(adst1_sb[:], 0.0)
        h1own_sb = const.tile([P, ntile, HC + 2 * H], BF16)
        nc.vector.memset(h1own_sb[:], 0.0)
        adst2e_sb = const.tile([P, nchunks], F32)
        t2keep = const.tile([P, ntile, R2E], F32)

        # ---- phase X: my nodes' h1 rows + a_dst1 (from xTmy) ----
        with tc.tile_pool(name="xpp", bufs=2, space="PSUM") as xpp:
            for t in range(ntile):
                rows = sched["tile_rows"][t]
                hp = xpp.tile([P, HC + 2 * H], F32, tag="hp")
                nc.tensor.matmul(out=hp[:rows, :],
                                 lhsT=xTmy_sb[:, t * P : t * P + rows],
                                 rhs=w1ev_sb[:], start=True, stop=True)
                nc.scalar.copy(h1own_sb[:rows, t, 0:HC], hp[:rows, 0:HC])
                nc.vector.tensor_copy(
                    h1own_sb[:rows, t, HC : HC + 2 * H].bitcast(F32),
                    hp[:rows, HC : HC + H])
                nc.vector.tensor_copy(adst1_sb[:rows, t, :],
                                      hp[:rows, HC + H : HC + 2 * H])

        # ---- phase A: T1 table (full graph, replicated; lo then hi) ----
        with tc.tile_pool(name="ap_", bufs=3) as apool, \
             tc.tile_pool(name="app", bufs=4, space="PSUM") as app:
            for i0 in range(0, natile, BAT):
                nb = min(BAT, natile - i0)
                xb = apool.tile([P, BAT, P], BF16, tag="xb")
                nc.sync.dma_start(
                    xb[:, 0:nb, :].rearrange("p b c -> p (b c)"),
                    i_xT[:, i0 * P : (i0 + nb) * P])
                t1r = apool.tile([P, BAT, HC + 16], BF16, tag="t1r")
                for b in range(nb):
                    hp = app.tile([P, HC + H], F32, tag="hp")
                    nc.tensor.matmul(out=hp[:], lhsT=xb[:, b, :],
                                     rhs=w1ev_sb[:, 0 : HC + H],
                                     start=True, stop=True)
                    if b % 2 == 0:
                        nc.scalar.copy(t1r[:, b, 0:HC], hp[:, 0:HC])
                    else:
                        nc.vector.tensor_copy(t1r[:, b, 0:HC], hp[:, 0:HC])
                    nc.vector.tensor_copy(
                        t1r[:, b, HC : HC + 16].bitcast(F32),
                        hp[:, HC : HC + H])
                dt, r0 = (d_t1lo, i0 * P) if i0 < nlo_t else \
                         (d_t1hi, (i0 - nlo_t) * P)
                nc.sync.dma_start(
                    dt[r0 : r0 + nb * P, 0 : HC + 16].rearrange(
                        "(b p) c -> p b c", p=P),
                    t1r[:, 0:nb, :])

        # ---- phase B: layer-1 aggregation + dense layer 2 ----
        with tc.tile_pool(name="g1", bufs=2) as g1p, \
             tc.tile_pool(name="sc1", bufs=3) as scp, \
             tc.tile_pool(name="gc1", bufs=2) as gcp, \
             tc.tile_pool(name="se1", bufs=4) as sep, \
             tc.tile_pool(name="agg1", bufs=2, space="PSUM") as aggp, \
             tc.tile_pool(name="adp1", bufs=2, space="PSUM") as adp, \
             tc.tile_pool(name="tp1", bufs=2, space="PSUM") as tpp, \
             tc.tile_pool(name="h2p", bufs=2, space="PSUM") as h2pp:
            for gi in sched["groups"]:
                c0, cl, chg, ct = gi["c0"], gi["clow"], gi["chigh"], gi["ct"]
                s0 = c0 * P
                idx_t = gcp.tile([P, ct * 8], I16, tag="idx")
                nc.sync.dma_start(idx_t[:], i_idx[:, c0 * 8 : (c0 + ct) * 8])
                dcol = gcp.tile([P, ct], F32, tag="dcol")
                nc.sync.dma_start(dcol[:], i_dstcol[:, c0 : c0 + ct])
                G = g1p.tile([P, ct, ROW1], BF16, tag="G")
                for (off, num, dt_) in ((0, cl, d_t1lo), (cl, chg, d_t1hi)):
                    if num == 0:
                        continue
                    nc.gpsimd.dma_gather(
                        out_ap=G[:, off : off + num, :],
                        in_ap=dt_[:, :],
                        idxs_ap=idx_t[:, off * 8 : (off + num) * 8],
                        num_idxs=num * P, num_idxs_reg=num * P,
                        elem_size=ROW1, single_packet=False)
                bc = gcp.tile([P, ct * P], BF16, tag="bc")
                nc.gpsimd.dma_start(
                    bc[:], i_dstrow[:, s0 : s0 + ct * P].to_broadcast(
                        [P, ct * P]))
                seT = gcp.tile([P, ct * P], BF16, tag="seT")
                nc.vector.tensor_tensor(
                    out=seT[:], in0=bc[:],
                    in1=iota_col[:, 0:1].to_broadcast([P, ct * P]),
                    op=OP.is_equal)
                seA = gcp.tile([P, ct * P], BF16, tag="seA")
                nc.vector.tensor_tensor(
                    out=seA[:].rearrange("p (c q) -> p c q", q=P),
                    in0=iota_rep[:, 0 : ct * P].rearrange(
                        "p (c q) -> p c q", q=P),
                    in1=dcol[:].rearrange("p (c o) -> p c o", o=1)
                        .to_broadcast([P, ct, P]),
                    op=OP.is_equal)
                ad_ps = adp.tile([P, ct * (H + 1)], F32, tag="ad")
                for c in range(ct):
                    t = sched["chunk_tile"][c0 + c]
                    nc.tensor.matmul(
                        out=ad_ps[:, c * (H + 1) : c * (H + 1) + H],
                        lhsT=seT[:, c * P : (c + 1) * P],
                        rhs=adst1_sb[:, t, :], start=True, stop=True)
                alpha = gcp.tile([P, ct * H], F32, tag="alpha")
                nc.vector.tensor_tensor(
                    out=alpha[:].rearrange("p (c h) -> p c h", h=H),
                    in0=G[:, :, HC : HC + 16].bitcast(F32),
                    in1=ad_ps[:].rearrange("p (c z) -> p c z", z=H + 1)
                        [:, :, 0:H],
                    op=OP.add)
                asc = gcp.tile([P, ct * H], F32, tag="asc")
                nc.vector.tensor_scalar(out=asc[:], in0=alpha[:],
                                        scalar1=NEG_SLOPE, scalar2=None,
                                        op0=OP.mult)
                nc.vector.tensor_tensor(out=alpha[:], in0=alpha[:], in1=asc[:],
                                        op=OP.max)
                w_sb = gcp.tile([P, ct * H], BF16, tag="w")
                nc.scalar.activation(w_sb[:], alpha[:], AF.Exp)
                nc.vector.tensor_copy(
                    G[:, :, HC : HC + H],
                    w_sb[:].rearrange("p (c h) -> p c h", h=H))
                g4 = G[:, :, 0:HC].rearrange("p c (h f) -> p c h f", f=C)
                wb = G[:, :, HC : HC + H].rearrange(
                    "p c (h o) -> p c h o", o=1).to_broadcast([P, ct, H, C])
                nc.vector.tensor_tensor(out=g4, in0=g4, in1=wb, op=OP.mult)

                for t in gi["tiles"]:
                    rows = sched["tile_rows"][t]
                    agg = aggp.tile([P, HC + H], F32, tag="agg")
                    tl, tlc = sched["tile_info"][t]["low"]
                    th, thc = sched["tile_info"][t]["high"]
                    tchunks = [tl - c0 + i_ for i_ in range(tlc)] + \
                              [th - c0 + i_ for i_ in range(thc)]
                    for j, c in enumerate(tchunks):
                        nc.tensor.matmul(
                            out=agg[:], lhsT=seA[:, c * P : (c + 1) * P],
                            rhs=G[:, c, 0 : HC + H],
                            start=(j == 0), stop=(j == len(tchunks) - 1))
                    # self-loop weight
                    aown = scp.tile([P, H], F32, tag="aown")
                    nc.vector.tensor_tensor(
                        out=aown[:],
                        in0=h1own_sb[:, t, HC : HC + 2 * H].bitcast(F32),
                        in1=adst1_sb[:, t, :], op=OP.add)
                    asn = scp.tile([P, H], F32, tag="asn")
                    nc.vector.tensor_scalar(out=asn[:], in0=aown[:],
                                            scalar1=NEG_SLOPE, scalar2=None,
                                            op0=OP.mult)
                    nc.vector.tensor_tensor(out=aown[:], in0=aown[:],
                                            in1=asn[:], op=OP.max)
                    wself = scp.tile([P, H], F32, tag="wself")
                    nc.scalar.activation(wself[:], aown[:], AF.Exp)
                    dn = scp.tile([P, H], F32, tag="dn")
                    nc.vector.tensor_tensor(out=dn[:], in0=agg[:, HC : HC + H],
                                            in1=wself[:], op=OP.add)
                    nc.vector.tensor_scalar(out=dn[:], in0=dn[:],
                                            scalar1=EPS, scalar2=None,
                                            op0=OP.add)
                    rc = scp.tile([P, H], F32, tag="rc")
                    nc.vector.reciprocal(rc[:], dn[:])
                    u = scp.tile([P, HC], F32, tag="u")
                    nc.vector.tensor_tensor(
                        out=u[:].rearrange("p (h f) -> p h f", f=C),
                        in0=h1own_sb[:, t, 0:HC].rearrange(
                            "p (h f) -> p h f", f=C),
                        in1=wself[:].rearrange("p (h o) -> p h o", o=1)
                            .to_broadcast([P, H, C]),
                        op=OP.mult)
                    nc.vector.tensor_tensor(out=u[:], in0=u[:],
                                            in1=agg[:, 0:HC], op=OP.add)
                    nc.vector.tensor_tensor(
                        out=u[:].rearrange("p (h f) -> p h f", f=C),
                        in0=u[:].rearrange("p (h f) -> p h f", f=C),
                        in1=rc[:].rearrange("p (h o) -> p h o", o=1)
                            .to_broadcast([P, H, C]),
                        op=OP.mult)
                    nc.vector.tensor_tensor(out=u[:], in0=u[:], in1=b1_sb[:],
                                            op=OP.add)
                    xm = scp.tile([P, HC], F32, tag="xm")
                    nc.vector.tensor_scalar(out=xm[:], in0=u[:], scalar1=0.0,
                                            scalar2=None, op0=OP.min)
                    em = scp.tile([P, HC], F32, tag="em")
                    nc.scalar.activation(em[:], xm[:], AF.Exp)
                    nc.vector.tensor_scalar(out=u[:], in0=u[:], scalar1=0.0,
                                            scalar2=None, op0=OP.max)
                    nc.vector.tensor_tensor(out=u[:], in0=u[:], in1=em[:],
                                            op=OP.add)
                    ub = scp.tile([P, HC], BF16, tag="ub")
                    nc.vector.tensor_copy(ub[:], u[:])
                    h2 = h2pp.tile([P, R2E], F32, tag="h2")
                    for j in range(2):
                        tp = tpp.tile([P, P], BF16, tag="tp")
                        nc.tensor.matmul(out=tp[:],
                                         lhsT=ub[:, j * P : (j + 1) * P],
                                         rhs=identbf[:], is_transpose=True,
                                         start=True, stop=True)
                        uT = sep.tile([P, P], BF16, tag="uT")
                        nc.vector.tensor_copy(uT[:], tp[:])
                        nc.tensor.matmul(out=h2[:], lhsT=uT[:],
                                         rhs=r2_sb[:, j, :],
                                         start=(j == 0), stop=(j == 1))
                    t2f = scp.tile([P, R2E], F32, tag="t2f")
                    nc.vector.tensor_tensor(out=t2f[:], in0=h2[:],
                                            in1=csum_sb[:], op=OP.subtract)
                    nc.vector.tensor_copy(t2keep[:, t, :], t2f[:])
                    t2w = scp.tile([P, R2E], BF16, tag="t2w")
                    nc.scalar.copy(t2w[:, 0:C], t2f[:, 0:C])
                    nc.vector.tensor_copy(t2w[:, C : C + 2].bitcast(F32),
                                          t2f[:, C : C + 1])
                    nc.sync.dma_start(d_t2s[t * P : t * P + rows, 0:R2E],
                                      t2w[:rows, :])
                    t2b = scp.tile([P, 1], BF16, tag="t2b")
                    nc.vector.tensor_copy(t2b[:], t2f[:, R2E - 1 : R2E])
                    for c in tchunks:
                        nc.tensor.matmul(
                            out=ad_ps[:, c * (H + 1) + H : (c + 1) * (H + 1)],
                            lhsT=seT[:, c * P : (c + 1) * P],
                            rhs=t2b[:], start=True, stop=True)
                nc.vector.tensor_copy(
                    adst2e_sb[:, c0 : c0 + ct].rearrange(
                        "p (c o) -> p c o", o=1),
                    ad_ps[:].rearrange("p (c z) -> p c z", z=H + 1)
                        [:, :, H : H + 1])

        # ---- collective: share T2 slabs ----
        nc.gpsimd.collective_compute(
            "AllGather", OP.bypass, replica_groups=[list(range(ncore))],
            ins=[d_t2s[:, :]], outs=[d_t2f[:, :]])

        # ---- phase D: layer-2 aggregation ----
        with tc.tile_pool(name="g2", bufs=2) as g2p, \
             tc.tile_pool(name="sc2", bufs=3) as sc2, \
             tc.tile_pool(name="gc2", bufs=2) as gc2, \
             tc.tile_pool(name="agg2", bufs=3, space="PSUM") as agg2p:
            for gi in sched["groups"]:
                c0, cl, chg, ct = gi["c0"], gi["clow"], gi["chigh"], gi["ct"]
                idx_t = gc2.tile([P, ct * 8], I16, tag="idx2")
                nc.sync.dma_start(idx_t[:], i_idx[:, c0 * 8 : (c0 + ct) * 8])
                dcol = gc2.tile([P, ct], F32, tag="dcol2")
                nc.sync.dma_start(dcol[:], i_dstcol[:, c0 : c0 + ct])
                G2 = g2p.tile([P, ct, ROW2], BF16, tag="G2")
                for (off, num, b0, b1_) in ((0, cl, 0, half),
                                            (cl, chg, half, n)):
                    if num == 0:
                        continue
                    nc.gpsimd.dma_gather(
                        out_ap=G2[:, off : off + num, :],
                        in_ap=d_t2f[b0:b1_, :],
                        idxs_ap=idx_t[:, off * 8 : (off + num) * 8],
                        num_idxs=num * P, num_idxs_reg=num * P,
                        elem_size=ROW2, single_packet=False)
                alpha = gc2.tile([P, ct], F32, tag="alpha2")
                nc.vector.tensor_tensor(
                    out=alpha[:].rearrange("p (c o) -> p c o", o=1),
                    in0=G2[:, :, C : C + 2].bitcast(F32),
                    in1=adst2e_sb[:, c0 : c0 + ct].rearrange(
                        "p (c o) -> p c o", o=1),
                    op=OP.add)
                asc = gc2.tile([P, ct], F32, tag="asc2")
                nc.vector.tensor_scalar(out=asc[:], in0=alpha[:],
                                        scalar1=NEG_SLOPE, scalar2=None,
                                        op0=OP.mult)
                nc.vector.tensor_tensor(out=alpha[:], in0=alpha[:], in1=asc[:],
                                        op=OP.max)
                w2 = gc2.tile([P, ct], BF16, tag="w2")
                nc.scalar.activation(w2[:], alpha[:], AF.Exp)
                nc.vector.tensor_copy(
                    G2[:, :, C : C + 1],
                    w2[:].rearrange("p (c o) -> p c o", o=1))
                nc.vector.tensor_tensor(
                    out=G2[:, :, 0:C], in0=G2[:, :, 0:C],
                    in1=G2[:, :, C : C + 1].to_broadcast([P, ct, C]),
                    op=OP.mult)
                seA = gc2.tile([P, ct * P], BF16, tag="seA2")
                nc.vector.tensor_tensor(
                    out=seA[:].rearrange("p (c q) -> p c q", q=P),
                    in0=iota_rep[:, 0 : ct * P].rearrange(
                        "p (c q) -> p c q", q=P),
                    in1=dcol[:].rearrange("p (c o) -> p c o", o=1)
                        .to_broadcast([P, ct, P]),
                    op=OP.is_equal)
                for t in gi["tiles"]:
                    rows = sched["tile_rows"][t]
                    agg = agg2p.tile([P, C + 1], F32, tag="agg2")
                    tl, tlc = sched["tile_info"][t]["low"]
                    th, thc = sched["tile_info"][t]["high"]
                    tchunks = [tl - c0 + i_ for i_ in range(tlc)] + \
                              [th - c0 + i_ for i_ in range(thc)]
                    for j, c in enumerate(tchunks):
                        nc.tensor.matmul(
                            out=agg[:], lhsT=seA[:, c * P : (c + 1) * P],
                            rhs=G2[:, c, 0 : C + 1],
                            start=(j == 0), stop=(j == len(tchunks) - 1))
                    a2o = sc2.tile([P, 1], F32, tag="a2o")
                    nc.vector.tensor_tensor(
                        out=a2o[:], in0=t2keep[:, t, C : C + 1],
                        in1=t2keep[:, t, C + 1 : C + 2], op=OP.add)
                    a2s = sc2.tile([P, 1], F32, tag="a2s")
                    nc.vector.tensor_scalar(out=a2s[:], in0=a2o[:],
                                            scalar1=NEG_SLOPE, scalar2=None,
                                            op0=OP.mult)
                    nc.vector.tensor_tensor(out=a2o[:], in0=a2o[:], in1=a2s[:],
                                            op=OP.max)
                    w2s = sc2.tile([P, 1], F32, tag="w2s")
                    nc.scalar.activation(w2s[:], a2o[:], AF.Exp)
                    dn = sc2.tile([P, 1], F32, tag="dn2")
                    nc.vector.tensor_tensor(out=dn[:], in0=agg[:, C : C + 1],
                                            in1=w2s[:], op=OP.add)
                    nc.vector.tensor_scalar(out=dn[:], in0=dn[:], scalar1=EPS,
                                            scalar2=None, op0=OP.add)
                    rc = sc2.tile([P, 1], F32, tag="rc2")
                    nc.vector.reciprocal(rc[:], dn[:])
                    ob = sc2.tile([P, C], F32, tag="ob")
                    nc.vector.tensor_tensor(
                        out=ob[:], in0=t2keep[:, t, 0:C],
                        in1=w2s[:].to_broadcast([P, C]), op=OP.mult)
                    nc.vector.tensor_tensor(out=ob[:], in0=ob[:],
                                            in1=agg[:, 0:C], op=OP.add)
                    nc.vector.tensor_tensor(out=ob[:], in0=ob[:],
                                            in1=rc[:].to_broadcast([P, C]),
                                            op=OP.mult)
                    nc.vector.tensor_tensor(out=ob[:], in0=ob[:], in1=b2_sb[:],
                                            op=OP.add)
                    nc.sync.dma_start(o_out[t * P : t * P + rows, :],
                                      ob[:rows, :])

    nc.compile()
    return nc


# ---------------------------------------------------------------- entry
_CACHE = {}


def _run(inputs, p):
    x = np.asarray(inputs["x"], np.float32)
    ei = np.asarray(inputs["edge_index"])
    n, ncore = p["n"], p["ncore"]
    sh = n // ncore
    natile = math.ceil(n / P)
    npos = natile * P

    sched, percore, nodes_at, pos_of = build_schedule(
        np.asarray(ei[0], np.int64), np.asarray(ei[1], np.int64), p)
    key = (sched["nchunks"], tuple(sched["csched"].reshape(-1).tolist()))
    if key not in _CACHE:
        _CACHE.clear()
        _CACHE[key] = build_program(sched, p)
    nc = _CACHE[key]

    w1ev, r2ext, csum, b1p = prep_weights(
        np.asarray(inputs["W1"], np.float32),
        np.asarray(inputs["att_src1"], np.float32),
        np.asarray(inputs["att_dst1"], np.float32),
        np.asarray(inputs["bias1"], np.float32),
        np.asarray(inputs["W2"], np.float32),
        np.asarray(inputs["att_src2"], np.float32),
        np.asarray(inputs["att_dst2"], np.float32), p)
    b2 = np.asarray(inputs["bias2"], np.float32)[None, :]

    xpos = x[nodes_at]                                  # [n, 128]
    xT = np.zeros((p["in_ch"], npos), BF)
    xT[:, 0:n] = np.ascontiguousarray(xpos.T).astype(BF)

    in_maps = []
    for k in range(ncore):
        in_maps.append(dict(
            xT=xT,
            xTmy=np.ascontiguousarray(xT[:, k * sh : (k + 1) * sh]),
            idx16=percore[k]["idx16"], dstcol=percore[k]["dstcol"],
            dstrow=percore[k]["dstrow"], w1ev=w1ev, r2ext=r2ext,
            csum=csum, bias1=b1p, bias2=b2))
    res = run_bass_kernel_spmd(nc, in_maps, core_ids=list(range(ncore)),
                               **p.get("run_kwargs", {}))
    out_pos = np.concatenate([res.results[k]["out"] for k in range(ncore)],
                             axis=0)
    out = np.empty_like(out_pos)
    out[nodes_at] = out_pos
    return out, res


def kernel(**inputs) -> np.ndarray:
    out, _ = _run(inputs, REAL)
    return out



# revision 3
# speedup vs baseline: 1.0004x; 1.0004x over previous
"""2-layer GAT (nn_GATModel) on 8 Trainium2 NeuronCores — v2.

Strategy (v2)
-------------
- Dst-node sharding with HOST-BALANCED node->(core,tile) assignment (LPT on
  in-degree) to minimize chunk padding; output inverse-permuted on host.
- Self-loop edges never gathered: their contribution is computed from
  locally-recomputed h1/h2 rows (phase X / t2keep) and added after
  aggregation.
- Phase A consumes host-transposed bf16 x (no on-device transposes), runs
  bf16 matmuls, and writes T1 rows (256 feat bf16 + 8 a_src f32, stride
  768B) in 8-tile batched DMAs.  T1 split into lo/hi DRAM tensors so the
  first low-half gathers can start while high tiles still compute.
- Phase B: per-edge rows gathered by SWDGE (idx16 lo/hi), one-hot Se/SeT
  built in ONE vector op per group, per-chunk PE matmuls for a_dst and
  aggregation (+ softmax denominator via a w column), dense layer-2
  transform fused per tile.
- Phase D: same structure on the AllGathered T2 table (256B rows).
"""

import sys

sys.path.insert(0, "/opt/trn_rl_repo")

import heapq
import math
from contextlib import ExitStack

import numpy as np
import ml_dtypes

import concourse.bass as bass  # noqa: F401
import concourse.tile as tile
from concourse import bacc, mybir
from concourse.bass_utils import run_bass_kernel_spmd
from concourse.masks import make_identity

P = 128
NEG_SLOPE = 0.2
EPS = 1e-16
F32 = mybir.dt.float32
BF16 = mybir.dt.bfloat16
I16 = mybir.dt.int16
BF = ml_dtypes.bfloat16

REAL = dict(
    n=50000, ncore=8, half=32768, heads=8, cdim=32, in_ch=128,
    cap_chunks=40, row1=384, row2=128,
)


# ---------------------------------------------------------------- host prep
def assign_nodes(deg, n, ncore, sh):
    """LPT-balance nodes into (core, tile) bins; return nodes_at, pos_of."""
    ntile = math.ceil(sh / P)
    caps = [P] * (ntile - 1) + [sh - P * (ntile - 1)]
    nbins = ncore * ntile
    order = np.argsort(-deg, kind="stable")
    heap = [(0, b) for b in range(nbins)]
    heapq.heapify(heap)
    fill = [[] for _ in range(nbins)]
    stash = []
    for v in order:
        while True:
            s, b = heapq.heappop(heap)
            if len(fill[b]) < caps[b % ntile]:
                break
            stash.append(None)  # bin full; drop permanently
        fill[b].append(v)
        if len(fill[b]) < caps[b % ntile]:
            heapq.heappush(heap, (s + int(deg[v]), b))
    nodes_at = np.empty(n, np.int64)
    pos_of = np.empty(n, np.int64)
    for b in range(nbins):
        k, t = divmod(b, ntile)
        base = k * sh + t * P
        for i, v in enumerate(fill[b]):
            nodes_at[base + i] = v
            pos_of[v] = base + i
    return nodes_at, pos_of, ntile


def build_schedule(src, dst, p):
    n, ncore, half = p["n"], p["ncore"], p["half"]
    sh = n // ncore
    deg = np.bincount(dst, minlength=n)
    nodes_at, pos_of, ntile = assign_nodes(deg, n, ncore, sh)

    sp = pos_of[src]
    dp = pos_of[dst]
    core = dp // sh
    within = dp % sh
    tl = within >> 7
    dloc = within & 127
    hi = (sp >= half).astype(np.int64)
    key = ((core * ntile + tl) * 2 + hi)
    order = np.argsort(key, kind="stable")
    sp, key = sp[order], key[order]
    dloc_s = dloc[order]
    hi_s = hi[order]

    bounds = np.searchsorted(key, np.arange(ncore * ntile * 2 + 1))
    counts = (bounds[1:] - bounds[:-1]).reshape(ncore, ntile, 2)
    cmax = counts.max(axis=0)                       # [ntile, 2]
    csched = np.maximum(-(-cmax // P), 1)

    groups = []
    cur = []
    for t in range(ntile):
        ct = int(csched[t].sum())
        if cur and sum(int(csched[x].sum()) for x in cur) + ct > p["cap_chunks"]:
            groups.append(cur)
            cur = []
        cur.append(t)
    groups.append(cur)

    chunk_tile = []
    group_info = []
    tile_info = {t: {} for t in range(ntile)}
    c0 = 0
    for g in groups:
        glow = sum(int(csched[t][0]) for t in g)
        ghigh = sum(int(csched[t][1]) for t in g)
        gi = dict(tiles=g, c0=c0, clow=glow, chigh=ghigh, ct=glow + ghigh)
        cc = c0
        for half_id in (0, 1):
            for t in g:
                nch = int(csched[t][half_id])
                tile_info[t]["low" if half_id == 0 else "high"] = (cc, nch)
                for _ in range(nch):
                    chunk_tile.append(t)
                cc += nch
        c0 = cc
        group_info.append(gi)
    nchunks = c0

    sched = dict(
        sh=sh, ntile=ntile, csched=csched, groups=group_info,
        chunk_tile=np.array(chunk_tile), tile_info=tile_info, nchunks=nchunks,
        tile_rows=[min(P, sh - t * P) for t in range(ntile)],
    )

    percore = []
    for k in range(ncore):
        idxs = np.zeros(nchunks * P, np.int16)
        dla = np.full(nchunks * P, 999.0, np.float32)
        for t in range(ntile):
            for half_id, kkey in ((0, "low"), (1, "high")):
                b0 = bounds[(k * ntile + t) * 2 + half_id]
                b1 = bounds[(k * ntile + t) * 2 + half_id + 1]
                vals = sp[b0:b1] - (half if half_id else 0)
                dv = dloc_s[b0:b1]
                cstart, cnum = tile_info[t][kkey]
                s0 = cstart * P
                idxs[s0 : s0 + len(vals)] = vals.astype(np.int16)
                dla[s0 : s0 + len(vals)] = dv.astype(np.float32)
        idx16 = np.tile(np.ascontiguousarray(idxs.reshape(-1, 16).T), (8, 1))
        dstcol = np.ascontiguousarray(
            dla.reshape(nchunks, P).T.astype(np.float32))
        dstrow = dla.reshape(1, -1).astype(BF)
        percore.append(dict(idx16=idx16, dstcol=dstcol, dstrow=dstrow))
    return sched, percore, nodes_at, pos_of


def prep_weights(W1, att_src1, att_dst1, bias1, W2, att_src2, att_dst2, p):
    H, C = p["heads"], p["cdim"]
    HC = H * C
    W1p = W1
    vsrc1 = np.einsum("khc,hc->kh", W1.reshape(-1, H, C), att_src1)
    vdst1 = np.einsum("khc,hc->kh", W1.reshape(-1, H, C), att_dst1)
    w1ev = np.concatenate([W1p, vsrc1, vdst1], axis=1)       # [128, 272]

    vsrc2 = W2 @ att_src2[0]
    vdst2 = W2 @ att_dst2[0]
    r2ext = np.concatenate(
        [W2, vsrc2[:, None], vdst2[:, None]], axis=1)        # [256, 34]
    r2b = r2ext.astype(BF)
    # csum must match the QUANTIZED weights: the elu+1 encoding relies on
    # exact cancellation of the +1 term (sum over 256 rows).
    csum = r2b.astype(np.float32).sum(axis=0)
    return (w1ev.astype(BF), r2b,
            csum.astype(np.float32)[None, :],
            bias1.astype(np.float32)[None, :])


# ---------------------------------------------------------------- device
def build_program(sched, p):
    n, ncore, half = p["n"], p["ncore"], p["half"]
    H, C, IC = p["heads"], p["cdim"], p["in_ch"]
    HC = H * C                                      # 256
    ROW1, ROW2 = p["row1"], p["row2"]               # 384, 128 (bf16 units)
    R2E = C + 2                                     # 34
    sh, ntile, nchunks = sched["sh"], sched["ntile"], sched["nchunks"]
    cap = p["cap_chunks"]
    natile = math.ceil(n / P)                       # 391
    nlo_t = half // P                               # 256 tiles in lo
    npos = natile * P                               # 50048
    BAT = 8

    nc = bacc.Bacc("TRN2", target_bir_lowering=False, debug=False,
                   num_devices=ncore)

    i_xT = nc.dram_tensor("xT", [IC, npos], BF16, kind="ExternalInput")
    i_xTmy = nc.dram_tensor("xTmy", [IC, sh], BF16, kind="ExternalInput")
    i_idx = nc.dram_tensor("idx16", [P, nchunks * 8], I16, kind="ExternalInput")
    i_dstcol = nc.dram_tensor("dstcol", [P, nchunks], F32, kind="ExternalInput")
    i_dstrow = nc.dram_tensor("dstrow", [1, nchunks * P], BF16,
                              kind="ExternalInput")
    i_w1ev = nc.dram_tensor("w1ev", [IC, HC + 2 * H], BF16,
                            kind="ExternalInput")
    i_r2 = nc.dram_tensor("r2ext", [HC, R2E], BF16, kind="ExternalInput")
    i_csum = nc.dram_tensor("csum", [1, R2E], F32, kind="ExternalInput")
    i_b1 = nc.dram_tensor("bias1", [1, HC], F32, kind="ExternalInput")
    i_b2 = nc.dram_tensor("bias2", [1, C], F32, kind="ExternalInput")
    o_out = nc.dram_tensor("out", [sh, C], F32, kind="ExternalOutput")

    d_t1lo = nc.dram_tensor("T1lo", [half, ROW1], BF16)
    d_t1hi = nc.dram_tensor("T1hi", [npos - half, ROW1], BF16)
    d_t2s = nc.dram_tensor("T2slab", [sh, ROW2], BF16)
    d_t2f = nc.dram_tensor("T2full", [n, ROW2], BF16,
                           addr_space="Shared" if ncore > 4 else "Local")

    AF = mybir.ActivationFunctionType
    OP = mybir.AluOpType

    with tile.TileContext(nc) as tc, ExitStack() as ctx:
        const = ctx.enter_context(tc.tile_pool(name="const", bufs=1))

        iota_col = const.tile([P, 1], F32)
        nc.gpsimd.iota(iota_col[:], pattern=[[0, 1]], base=0,
                       channel_multiplier=1,
                       allow_small_or_imprecise_dtypes=True)
        iota_rep = const.tile([P, cap * P], BF16)
        nc.gpsimd.iota(iota_rep[:], pattern=[[0, cap], [1, P]], base=0,
                       channel_multiplier=0,
                       allow_small_or_imprecise_dtypes=True)
        ident32 = const.tile([P, P], F32)
        make_identity(nc, ident32[:])
        identbf = const.tile([P, P], BF16)
        make_identity(nc, identbf[:])
        w1ev_sb = const.tile([IC, HC + 2 * H], BF16)
        nc.sync.dma_start(w1ev_sb[:], i_w1ev[:, :])
        r2_sb = const.tile([P, 2, R2E], BF16)
        for j in range(2):
            nc.sync.dma_start(r2_sb[:, j, :], i_r2[j * P : (j + 1) * P, :])
        csum_sb = const.tile([P, R2E], F32)
        nc.gpsimd.dma_start(csum_sb[:], i_csum[:, :].to_broadcast([P, R2E]))
        b1_sb = const.tile([P, HC], F32)
        nc.gpsimd.dma_start(b1_sb[:], i_b1[:, :].to_broadcast([P, HC]))
        b2_sb = const.tile([P, C], F32)
        nc.gpsimd.dma_start(b2_sb[:], i_b2[:, :].to_broadcast([P, C]))
        xTmy_sb = const.tile([IC, sh], BF16)
        nc.sync.dma_start(xTmy_sb[:], i_xTmy[:, :])
        adst1_sb = const.tile([P, ntile, H], BF16)
        nc.vector.memset(adst1_sb[:], 0.0)
        h1own_sb = const.tile([P, ntile, HC + 2 * H], BF16)
        nc.vector.memset(h1own_sb[:], 0.0)
        adst2e_sb = const.tile([P, nchunks], F32)
        t2keep = const.tile([P, ntile, R2E], F32)

        # ---- phase X: my nodes' h1 rows + a_dst1 (from xTmy) ----
        with tc.tile_pool(name="xpp", bufs=2, space="PSUM") as xpp:
            for t in range(ntile):
                rows = sched["tile_rows"][t]
                hp = xpp.tile([P, HC + 2 * H], F32, tag="hp")
                nc.tensor.matmul(out=hp[:rows, :],
                                 lhsT=xTmy_sb[:, t * P : t * P + rows],
                                 rhs=w1ev_sb[:], start=True, stop=True)
                nc.scalar.copy(h1own_sb[:rows, t, 0:HC], hp[:rows, 0:HC])
                nc.vector.tensor_copy(
                    h1own_sb[:rows, t, HC : HC + 2 * H].bitcast(F32),
                    hp[:rows, HC : HC + H])
                nc.vector.tensor_copy(adst1_sb[:rows, t, :],
                                      hp[:rows, HC + H : HC + 2 * H])

        # ---- phase A: T1 table (full graph, replicated; lo then hi) ----
        with tc.tile_pool(name="ap_", bufs=3) as apool, \
             tc.tile_pool(name="app", bufs=4, space="PSUM") as app:
            for i0 in range(0, natile, BAT):
                nb = min(BAT, natile - i0)
                xb = apool.tile([P, BAT, P], BF16, tag="xb")
                nc.sync.dma_start(
                    xb[:, 0:nb, :].rearrange("p b c -> p (b c)"),
                    i_xT[:, i0 * P : (i0 + nb) * P])
                t1r = apool.tile([P, BAT, HC + 16], BF16, tag="t1r")
                for b in range(nb):
                    hp = app.tile([P, HC + H], F32, tag="hp")
                    nc.tensor.matmul(out=hp[:], lhsT=xb[:, b, :],
                                     rhs=w1ev_sb[:, 0 : HC + H],
                                     start=True, stop=True)
                    if b % 2 == 0:
                        nc.scalar.copy(t1r[:, b, 0:HC], hp[:, 0:HC])
                    else:
                        nc.vector.tensor_copy(t1r[:, b, 0:HC], hp[:, 0:HC])
                    nc.vector.tensor_copy(
                        t1r[:, b, HC : HC + 16].bitcast(F32),
                        hp[:, HC : HC + H])
                dt, r0 = (d_t1lo, i0 * P) if i0 < nlo_t else \
                         (d_t1hi, (i0 - nlo_t) * P)
                nc.sync.dma_start(
                    dt[r0 : r0 + nb * P, 0 : HC + 16].rearrange(
                        "(b p) c -> p b c", p=P),
                    t1r[:, 0:nb, :])

        # ---- phase B: layer-1 aggregation + dense layer 2 ----
        with tc.tile_pool(name="g1", bufs=2) as g1p, \
             tc.tile_pool(name="sc1", bufs=3) as scp, \
             tc.tile_pool(name="gc1", bufs=2) as gcp, \
             tc.tile_pool(name="se1", bufs=4) as sep, \
             tc.tile_pool(name="agg1", bufs=2, space="PSUM") as aggp, \
             tc.tile_pool(name="adp1", bufs=2, space="PSUM") as adp, \
             tc.tile_pool(name="tp1", bufs=2, space="PSUM") as tpp, \
             tc.tile_pool(name="h2p", bufs=2, space="PSUM") as h2pp:
            for gi in sched["groups"]:
                c0, cl, chg, ct = gi["c0"], gi["clow"], gi["chigh"], gi["ct"]
                s0 = c0 * P
                idx_t = gcp.tile([P, ct * 8], I16, tag="idx")
                nc.sync.dma_start(idx_t[:], i_idx[:, c0 * 8 : (c0 + ct) * 8])
                dcol = gcp.tile([P, ct], F32, tag="dcol")
                nc.sync.dma_start(dcol[:], i_dstcol[:, c0 : c0 + ct])
                G = g1p.tile([P, ct, ROW1], BF16, tag="G")
                for (off, num, dt_) in ((0, cl, d_t1lo), (cl, chg, d_t1hi)):
                    if num == 0:
                        continue
                    nc.gpsimd.dma_gather(
                        out_ap=G[:, off : off + num, :],
                        in_ap=dt_[:, :],
                        idxs_ap=idx_t[:, off * 8 : (off + num) * 8],
                        num_idxs=num * P, num_idxs_reg=num * P,
                        elem_size=ROW1, single_packet=False)
                bc = gcp.tile([P, ct * P], BF16, tag="bc")
                nc.gpsimd.dma_start(
                    bc[:], i_dstrow[:, s0 : s0 + ct * P].to_broadcast(
                        [P, ct * P]))
                seT = gcp.tile([P, ct * P], BF16, tag="seT")
                nc.vector.tensor_tensor(
                    out=seT[:], in0=bc[:],
                    in1=iota_col[:, 0:1].to_broadcast([P, ct * P]),
                    op=OP.is_equal)
                seA = gcp.tile([P, ct * P], BF16, tag="seA")
                nc.vector.tensor_tensor(
                    out=seA[:].rearrange("p (c q) -> p c q", q=P),
                    in0=iota_rep[:, 0 : ct * P].rearrange(
                        "p (c q) -> p c q", q=P),
                    in1=dcol[:].rearrange("p (c o) -> p c o", o=1)
                        .to_broadcast([P, ct, P]),
                    op=OP.is_equal)
                ad_ps = adp.tile([P, ct * (H + 1)], F32, tag="ad")
                for c in range(ct):
                    t = sched["chunk_tile"][c0 + c]
                    nc.tensor.matmul(
                        out=ad_ps[:, c * (H + 1) : c * (H + 1) + H],
                        lhsT=seT[:, c * P : (c + 1) * P],
                        rhs=adst1_sb[:, t, :], start=True, stop=True)
                alpha = gcp.tile([P, ct * H], F32, tag="alpha")
                nc.vector.tensor_tensor(
                    out=alpha[:].rearrange("p (c h) -> p c h", h=H),
                    in0=G[:, :, HC : HC + 16].bitcast(F32),
                    in1=ad_ps[:].rearrange("p (c z) -> p c z", z=H + 1)
                        [:, :, 0:H],
                    op=OP.add)
                asc = gcp.tile([P, ct * H], F32, tag="asc")
                nc.vector.tensor_scalar(out=asc[:], in0=alpha[:],
                                        scalar1=NEG_SLOPE, scalar2=None,
                                        op0=OP.mult)
                nc.vector.tensor_tensor(out=alpha[:], in0=alpha[:], in1=asc[:],
                                        op=OP.max)
                w_sb = gcp.tile([P, ct * H], BF16, tag="w")
                nc.scalar.activation(w_sb[:], alpha[:], AF.Exp)
                nc.vector.tensor_copy(
                    G[:, :, HC : HC + H],
                    w_sb[:].rearrange("p (c h) -> p c h", h=H))
                g4 = G[:, :, 0:HC].rearrange("p c (h f) -> p c h f", f=C)
                wb = G[:, :, HC : HC + H].rearrange(
                    "p c (h o) -> p c h o", o=1).to_broadcast([P, ct, H, C])
                nc.vector.tensor_tensor(out=g4, in0=g4, in1=wb, op=OP.mult)

                for t in gi["tiles"]:
                    rows = sched["tile_rows"][t]
                    agg = aggp.tile([P, HC + H], F32, tag="agg")
                    tl, tlc = sched["tile_info"][t]["low"]
                    th, thc = sched["tile_info"][t]["high"]
                    tchunks = [tl - c0 + i_ for i_ in range(tlc)] + \
                              [th - c0 + i_ for i_ in range(thc)]
                    for j, c in enumerate(tchunks):
                        nc.tensor.matmul(
                            out=agg[:], lhsT=seA[:, c * P : (c + 1) * P],
                            rhs=G[:, c, 0 : HC + H],
                            start=(j == 0), stop=(j == len(tchunks) - 1))
                    # self-loop weight
                    aown = scp.tile([P, H], F32, tag="aown")
                    nc.vector.tensor_tensor(
                        out=aown[:],
                        in0=h1own_sb[:, t, HC : HC + 2 * H].bitcast(F32),
                        in1=adst1_sb[:, t, :], op=OP.add)
                    asn = scp.tile([P, H], F32, tag="asn")
                    nc.vector.tensor_scalar(out=asn[:], in0=aown[:],
                                            scalar1=NEG_SLOPE, scalar2=None,
                                            op0=OP.mult)
                    nc.vector.tensor_tensor(out=aown[:], in0=aown[:],
                                            in1=asn[:], op=OP.max)
                    wself = scp.tile([P, H], F32, tag="wself")
                    nc.scalar.activation(wself[:], aown[:], AF.Exp)
                    dn = scp.tile([P, H], F32, tag="dn")
                    nc.vector.tensor_tensor(out=dn[:], in0=agg[:, HC : HC + H],
                                            in1=wself[:], op=OP.add)
                    nc.vector.tensor_scalar(out=dn[:], in0=dn[:],
                                            scalar1=EPS, scalar2=None,
                                            op0=OP.add)
                    rc = scp.tile([P, H], F32, tag="rc")
                    nc.vector.reciprocal(rc[:], dn[:])
                    u = scp.tile([P, HC], F32, tag="u")
                    nc.vector.tensor_tensor(
                        out=u[:].rearrange("p (h f) -> p h f", f=C),
                        in0=h1own_sb[:, t, 0:HC].rearrange(
                            "p (h f) -> p h f", f=C),
                        in1=wself[:].rearrange("p (h o) -> p h o", o=1)
                            .to_broadcast([P, H, C]),
                        op=OP.mult)
                    nc.vector.tensor_tensor(out=u[:], in0=u[:],
                                            in1=agg[:, 0:HC], op=OP.add)
                    nc.vector.tensor_tensor(
                        out=u[:].rearrange("p (h f) -> p h f", f=C),
                        in0=u[:].rearrange("p (h f) -> p h f", f=C),
                        in1=rc[:].rearrange("p (h o) -> p h o", o=1)
                            .to_broadcast([P, H, C]),
                        op=OP.mult)
                    nc.vector.tensor_tensor(out=u[:], in0=u[:], in1=b1_sb[:],
                                            op=OP.add)
                    xm = scp.tile([P, HC], F32, tag="xm")
                    nc.vector.tensor_scalar(out=xm[:], in0=u[:], scalar1=0.0,
                                            scalar2=None, op0=OP.min)
                    em = scp.tile([P, HC], F32, tag="em")
                    nc.scalar.activation(em[:], xm[:], AF.Exp)
                    nc.vector.tensor_scalar(out=u[:], in0=u[:], scalar1=0.0,
                                            scalar2=None, op0=OP.max)
                    nc.vector.tensor_tensor(out=u[:], in0=u[:], in1=em[:],
                                            op=OP.add)
                    ub = scp.tile([P, HC], BF16, tag="ub")
                    nc.vector.tensor_copy(ub[:], u[:])
                    h2 = h2pp.tile([P, R2E], F32, tag="h2")
                    for j in range(2):
                        tp = tpp.tile([P, P], BF16, tag="tp")
                        nc.tensor.matmul(out=tp[:],
                                         lhsT=ub[:, j * P : (j + 1) * P],
                                         rhs=identbf[:], is_transpose=True,
                                         start=True, stop=True)
                        uT = sep.tile([P, P], BF16, tag="uT")
                        nc.vector.tensor_copy(uT[:], tp[:])
                        nc.tensor.matmul(out=h2[:], lhsT=uT[:],
                                         rhs=r2_sb[:, j, :],
                                         start=(j == 0), stop=(j == 1))
                    t2f = scp.tile([P, R2E], F32, tag="t2f")
                    nc.vector.tensor_tensor(out=t2f[:], in0=h2[:],
                                            in1=csum_sb[:], op=OP.subtract)
                    nc.vector.tensor_copy(t2keep[:, t, :], t2f[:])
                    t2w = scp.tile([P, R2E], BF16, tag="t2w")
                    nc.scalar.copy(t2w[:, 0:C], t2f[:, 0:C])
                    nc.vector.tensor_copy(t2w[:, C : C + 2].bitcast(F32),
                                          t2f[:, C : C + 1])
                    nc.sync.dma_start(d_t2s[t * P : t * P + rows, 0:R2E],
                                      t2w[:rows, :])
                    t2b = scp.tile([P, 1], BF16, tag="t2b")
                    nc.vector.tensor_copy(t2b[:], t2f[:, R2E - 1 : R2E])
                    for c in tchunks:
                        nc.tensor.matmul(
                            out=ad_ps[:, c * (H + 1) + H : (c + 1) * (H + 1)],
                            lhsT=seT[:, c * P : (c + 1) * P],
                            rhs=t2b[:], start=True, stop=True)
                nc.vector.tensor_copy(
                    adst2e_sb[:, c0 : c0 + ct].rearrange(
                        "p (c o) -> p c o", o=1),
                    ad_ps[:].rearrange("p (c z) -> p c z", z=H + 1)
                        [:, :, H : H + 1])

        # ---- collective: share T2 slabs ----
        nc.gpsimd.collective_compute(
            "AllGather", OP.bypass, replica_groups=[list(range(ncore))],
            ins=[d_t2s[:, :]], outs=[d_t2f[:, :]])

        # ---- phase D: layer-2 aggregation ----
        with tc.tile_pool(name="g2", bufs=2) as g2p, \
             tc.tile_pool(name="sc2", bufs=3) as sc2, \
             tc.tile_pool(name="gc2", bufs=2) as gc2, \
             tc.tile_pool(name="agg2", bufs=3, space="PSUM") as agg2p:
            for gi in sched["groups"]:
                c0, cl, chg, ct = gi["c0"], gi["clow"], gi["chigh"], gi["ct"]
                idx_t = gc2.tile([P, ct * 8], I16, tag="idx2")
                nc.sync.dma_start(idx_t[:], i_idx[:, c0 * 8 : (c0 + ct) * 8])
                dcol = gc2.tile([P, ct], F32, tag="dcol2")
                nc.sync.dma_start(dcol[:], i_dstcol[:, c0 : c0 + ct])
                G2 = g2p.tile([P, ct, ROW2], BF16, tag="G2")
                for (off, num, b0, b1_) in ((0, cl, 0, half),
                                            (cl, chg, half, n)):
                    if num == 0:
                        continue
                    nc.gpsimd.dma_gather(
                        out_ap=G2[:, off : off + num, :],
                        in_ap=d_t2f[b0:b1_, :],
                        idxs_ap=idx_t[:, off * 8 : (off + num) * 8],
                        num_idxs=num * P, num_idxs_reg=num * P,
                        elem_size=ROW2, single_packet=False)
                alpha = gc2.tile([P, ct], F32, tag="alpha2")
                nc.vector.tensor_tensor(
                    out=alpha[:].rearrange("p (c o) -> p c o", o=1),
                    in0=G2[:, :, C : C + 2].bitcast(F32),
                    in1=adst2e_sb[:, c0 : c0 + ct].rearrange(
                        "p (c o) -> p c o", o=1),
                    op=OP.add)
                asc = gc2.tile([P, ct], F32, tag="asc2")
                nc.vector.tensor_scalar(out=asc[:], in0=alpha[:],
                                        scalar1=NEG_SLOPE, scalar2=None,
                                        op0=OP.mult)
                nc.vector.tensor_tensor(out=alpha[:], in0=alpha[:], in1=asc[:],
                                        op=OP.max)
                w2 = gc2.tile([P, ct], BF16, tag="w2")
                nc.scalar.activation(w2[:], alpha[:], AF.Exp)
                nc.vector.tensor_copy(
                    G2[:, :, C : C + 1],
                    w2[:].rearrange("p (c o) -> p c o", o=1))
                nc.vector.tensor_tensor(
                    out=G2[:, :, 0:C], in0=G2[:, :, 0:C],
                    in1=G2[:, :, C : C + 1].to_broadcast([P, ct, C]),
                    op=OP.mult)
                seA = gc2.tile([P, ct * P], BF16, tag="seA2")
                nc.vector.tensor_tensor(
                    out=seA[:].rearrange("p (c q) -> p c q", q=P),
                    in0=iota_rep[:, 0 : ct * P].rearrange(
                        "p (c q) -> p c q", q=P),
                    in1=dcol[:].rearrange("p (c o) -> p c o", o=1)
                        .to_broadcast([P, ct, P]),
                    op=OP.is_equal)
                for t in gi["tiles"]:
                    rows = sched["tile_rows"][t]
                    agg = agg2p.tile([P, C + 1], F32, tag="agg2")
                    tl, tlc = sched["tile_info"][t]["low"]
                    th, thc = sched["tile_info"][t]["high"]
                    tchunks = [tl - c0 + i_ for i_ in range(tlc)] + \
                              [th - c0 + i_ for i_ in range(thc)]
                    for j, c in enumerate(tchunks):
                        nc.tensor.matmul(
                            out=agg[:], lhsT=seA[:, c * P : (c + 1) * P],
                            rhs=G2[:, c, 0 : C + 1],
                            start=(j == 0), stop=(j == len(tchunks) - 1))
                    a2o = sc2.tile([P, 1], F32, tag="a2o")
                    nc.vector.tensor_tensor(
                        out=a2o[:], in0=t2keep[:, t, C : C + 1],
                        in1=t2keep[:, t, C + 1 : C + 2], op=OP.add)
                    a2s = sc2.tile([P, 1], F32, tag="a2s")
                    nc.vector.tensor_scalar(out=a2s[:], in0=a2o[:],
                                            scalar1=NEG_SLOPE, scalar2=None,
                                            op0=OP.mult)
                    nc.vector.tensor_tensor(out=a2o[:], in0=a2o[:], in1=a2s[:],
                                            op=OP.max)
                    w2s = sc2.tile([P, 1], F32, tag="w2s")
                    nc.scalar.activation(w2s[:], a2o[:], AF.Exp)
                    dn = sc2.tile([P, 1], F32, tag="dn2")
                    nc.vector.tensor_tensor(out=dn[:], in0=agg[:, C : C + 1],
                                            in1=w2s[:], op=OP.add)
                    nc.vector.tensor_scalar(out=dn[:], in0=dn[:], scalar1=EPS,
                                            scalar2=None, op0=OP.add)
                    rc = sc2.tile([P, 1], F32, tag="rc2")
                    nc.vector.reciprocal(rc[:], dn[:])
                    ob = sc2.tile([P, C], F32, tag="ob")
                    nc.vector.tensor_tensor(
                        out=ob[:], in0=t2keep[:, t, 0:C],
                        in1=w2s[:].to_broadcast([P, C]), op=OP.mult)
                    nc.vector.tensor_tensor(out=ob[:], in0=ob[:],
                                            in1=agg[:, 0:C], op=OP.add)
                    nc.vector.tensor_tensor(out=ob[:], in0=ob[:],
                                            in1=rc[:].to_broadcast([P, C]),
                                            op=OP.mult)
                    nc.vector.tensor_tensor(out=ob[:], in0=ob[:], in1=b2_sb[:],
                                            op=OP.add)
                    nc.sync.dma_start(o_out[t * P : t * P + rows, :],
                                      ob[:rows, :])

    nc.compile()
    return nc


# ---------------------------------------------------------------- entry
_CACHE = {}


def _run(inputs, p):
    x = np.asarray(inputs["x"], np.float32)
    ei = np.asarray(inputs["edge_index"])
    n, ncore = p["n"], p["ncore"]
    sh = n // ncore
    natile = math.ceil(n / P)
    npos = natile * P

    sched, percore, nodes_at, pos_of = build_schedule(
        np.asarray(ei[0], np.int64), np.asarray(ei[1], np.int64), p)
    key = (sched["nchunks"], tuple(sched["csched"].reshape(-1).tolist()))
    if key not in _CACHE:
        _CACHE.clear()
        _CACHE[key] = build_program(sched, p)
    nc = _CACHE[key]

    w1ev, r2ext, csum, b1p = prep_weights(
        np.asarray(inputs["W1"], np.float32),
        np.asarray(inputs["att_src1"], np.float32),
        np.asarray(inputs["att_dst1"], np.float32),
        np.asarray(inputs["bias1"], np.float32),
        np.asarray(inputs["W2"], np.float32),
        np.asarray(inputs["att_src2"], np.float32),
        np.asarray(inputs["att_dst2"], np.float32), p)
    b2 = np.asarray(inputs["bias2"], np.float32)[None, :]

    xpos = x[nodes_at]                                  # [n, 128]
    xT = np.zeros((p["in_ch"], npos), BF)
    xT[:, 0:n] = np.ascontiguousarray(xpos.T).astype(BF)

    in_maps = []
    for k in range(ncore):
        in_maps.append(dict(
            xT=xT,
            xTmy=np.ascontiguousarray(xT[:, k * sh : (k + 1) * sh]),
            idx16=percore[k]["idx16"], dstcol=percore[k]["dstcol"],
            dstrow=percore[k]["dstrow"], w1ev=w1ev, r2ext=r2ext,
            csum=csum, bias1=b1p, bias2=b2))
    res = run_bass_kernel_spmd(nc, in_maps, core_ids=list(range(ncore)),
                               **p.get("run_kwargs", {}))
    out_pos = np.concatenate([res.results[k]["out"] for k in range(ncore)],
                             axis=0)
    out = np.empty_like(out_pos)
    out[nodes_at] = out_pos
    return out, res


def kernel(**inputs) -> np.ndarray:
    out, _ = _run(inputs, REAL)
    return out

